# revision 13
# baseline (speedup 1.0000x reference)
"""Megatron-style MoE layer (precomputed routing) on 8 Trainium2 NeuronCores.

Strategy: expert parallelism. Core e owns expert e's weights (w1[e], w2[e],
resident in SBUF as bf16). The host computes the token->expert routing from
`choosed_experts` (pure index math), gathers each expert's tokens into a
padded, transposed [H, C] activation block, and each core computes

    y_e = coef * (gelu_tanh(x_e @ w1[e]) @ w2[e])

entirely on-device in a features-on-partition layout ([features, tokens]),
so both matmuls use the natural weight layout as lhsT and no on-chip
transposes are needed. The host scatters the per-pair results back and sums
the K=2 contributions per token.

Device layouts (per core), P = 128 partitions:
  xT   [P, 8, C]  bf16   x^T, h = ko*128 + p
  w1   [P, 8, F]  bf16   w1[h, f], h = ko*128 + p  (lhsT for fc1)
  w2   [P, 32, H] bf16   w2[f, hh], f = kf*128 + p (lhsT for fc2)
  coef [P, C]     f32    per-token gate prob, replicated across partitions
  y    [P, 8, C]  f32    y^T, hh = mh*128 + p
"""

import sys
import numpy as np
import ml_dtypes


def _ensure_axon_hooks():
    """bass_utils imports antenv.axon_hooks when BASS_TRACE is set; this
    image ships an antenv stub without it. Provide a working (or None)
    hook so tracing requests degrade gracefully instead of crashing."""
    try:
        import antenv.axon_hooks  # noqa: F401
        return
    except ImportError:
        pass
    import os
    import types

    mod = types.ModuleType("antenv.axon_hooks")
    state = [None]

    def set_axon_ntff_profile_hook(h):
        state[0] = h

    def get_axon_ntff_profile_hook():
        if state[0] is None:
            try:
                from trn_agent_boot.trn_boot import _ntff_profile_via_ctypes
                so = os.environ.get("PJRT_LIBRARY_PATH",
                                    "/opt/axon/libaxon_pjrt.so")
                if os.path.exists(so):
                    state[0] = _ntff_profile_via_ctypes(so)
            except Exception:
                pass
        return state[0]

    mod.set_axon_ntff_profile_hook = set_axon_ntff_profile_hook
    mod.get_axon_ntff_profile_hook = get_axon_ntff_profile_hook
    sys.modules["antenv.axon_hooks"] = mod
    try:
        import antenv
        antenv.axon_hooks = mod
    except ImportError:
        pass
    try:
        from concourse import bass_utils as _bu
        _orig = _bu.upload_artifacts

        def _safe_upload(tmpdir):
            try:
                return _orig(tmpdir)
            except Exception:
                return "local://" + tmpdir

        _bu.upload_artifacts = _safe_upload
    except Exception:
        pass


S, B, H = 1024, 8, 1024
T = S * B
E, K, F = 8, 2, 4096
P = 128
NCORES = 8

_CACHE: dict[int, object] = {}

TRACE = False
LAST_RESULTS = None


def _build(C: int):
    import concourse.bacc as bacc
    import concourse.mybir as mybir
    import concourse.tile as tile

    dt = mybir.dt
    AF = mybir.ActivationFunctionType

    nc = bacc.Bacc("TRN2", target_bir_lowering=False, debug=False,
                   num_devices=NCORES)

    xT_d = nc.dram_tensor("xT", [P, 8, C], dt.bfloat16, kind="ExternalInput").ap()
    w1_d = nc.dram_tensor("w1", [P, 8, F], dt.bfloat16, kind="ExternalInput").ap()
    w2_d = nc.dram_tensor("w2", [P, 32, H], dt.bfloat16, kind="ExternalInput").ap()
    cf_d = nc.dram_tensor("coef", [P, C], dt.float32, kind="ExternalInput").ap()
    y_d = nc.dram_tensor("y", [P, 8, C], dt.float32, kind="ExternalOutput").ap()

    # token tiles of up to 512 (PSUM bank limit for f32 output), sized as
    # evenly as possible (multiples of 32) so every tile stays in the
    # PE streaming regime (N >= 128) instead of one LDWEIGHTS-bound tail
    nt = -(-C // 512)
    base = C // nt
    sizes = []
    rem = C
    for i in range(nt):
        n = min(512, -(-rem // (nt - i)))
        n = -(-n // 32) * 32 if rem - n >= 32 or rem == n else rem
        n = min(n, rem)
        sizes.append(n)
        rem -= n
    assert sum(sizes) == C and all(0 < s <= 512 for s in sizes), sizes
    tiles = []
    n0 = 0
    for n in sizes:
        tiles.append((n0, n))
        n0 += n

    with tile.TileContext(nc) as tc:
        with (
            tc.tile_pool(name="wpool", bufs=1) as wpool,
            tc.tile_pool(name="xpool", bufs=2) as xpool,
            tc.tile_pool(name="hpool", bufs=1) as hpool,
            tc.tile_pool(name="opool", bufs=4) as opool,
            tc.tile_pool(name="ps1", bufs=3, space="PSUM") as ps1,
            tc.tile_pool(name="ps2", bufs=3, space="PSUM") as ps2,
        ):
            w1_sb = wpool.tile([P, 8, F], dt.bfloat16, tag="w1")
            w2_sb = wpool.tile([P, 32, H], dt.bfloat16, tag="w2")

            # All sync-engine DMAs share one in-order HWDGE queue, so issue
            # order = completion order. Load the first x tile and w1 first
            # (fc1's critical path), defer w2 until fc1 is underway.
            # The opening cascade is fine-grained and interleaved so the
            # first matmul group (mf=0: w1 f-cols 0:128 + all ko of x)
            # becomes runnable after ~0.6 MB instead of ~1.8 MB.
            N0 = tiles[0][1]
            xt0 = xpool.tile([P, 8, 512], dt.bfloat16, tag="x")
            nc.sync.dma_start(w1_sb[:, :, 0:128], w1_d[:, :, 0:128])
            nc.sync.dma_start(xt0[:, 0:2, :N0], xT_d[:, 0:2, :N0])
            nc.sync.dma_start(xt0[:, 2:4, :N0], xT_d[:, 2:4, :N0])
            nc.sync.dma_start(xt0[:, 4:8, :N0], xT_d[:, 4:8, :N0])
            # rest of w1, coarsening as the PE gets further ahead
            w1_chunks = [(128, 128), (256, 256), (512, 512)] + \
                        [(i * 512, 512) for i in range(2, 8)]
            for (f0, fn) in w1_chunks:
                nc.sync.dma_start(w1_sb[:, :, f0:f0 + fn],
                                  w1_d[:, :, f0:f0 + fn])

            for ti, (t0, N) in enumerate(tiles):
                if ti == 0:
                    xt = xt0
                else:
                    xt = xpool.tile([P, 8, 512], dt.bfloat16, tag="x")
                    nc.sync.dma_start(xt[:, :, :N], xT_d[:, :, t0:t0 + N])
                cf = xpool.tile([P, 512], dt.float32, tag="cf")
                nc.sync.dma_start(cf[:, :N], cf_d[:, t0:t0 + N])

                h = hpool.tile([P, 32, 512], dt.bfloat16, tag="h")
                for mf in range(32):
                    p1 = ps1.tile([P, 512], dt.float32, tag="p1")
                    for ko in range(8):
                        nc.tensor.matmul(
                            p1[:, :N],
                            w1_sb[:, ko, mf * 128:(mf + 1) * 128],
                            xt[:, ko, :N],
                            start=(ko == 0), stop=(ko == 7),
                        )
                    nc.scalar.activation(h[:, mf, :N], p1[:, :N],
                                         AF.Gelu_apprx_tanh)

                if ti == 0:
                    # w2 isn't needed until fc2 of tile 0; issuing it here
                    # keeps it off fc1's DMA critical path
                    for i in range(8):
                        nc.sync.dma_start(w2_sb[:, i * 4:(i + 1) * 4, :],
                                          w2_d[:, i * 4:(i + 1) * 4, :])

                for mh in range(8):
                    p2 = ps2.tile([P, 512], dt.float32, tag="p2")
                    for kf in range(32):
                        nc.tensor.matmul(
                            p2[:, :N],
                            w2_sb[:, kf, mh * 128:(mh + 1) * 128],
                            h[:, kf, :N],
                            start=(kf == 0), stop=(kf == 31),
                        )
                    ot = opool.tile([P, 512], dt.float32, tag="o")
                    nc.vector.tensor_mul(ot[:, :N], p2[:, :N], cf[:, :N])
                    nc.sync.dma_start(y_d[:, mh, t0:t0 + N], ot[:, :N])

    nc.compile()
    return nc


def kernel(hidden_states, gate_weight, choosed_experts, w1, w2):
    global LAST_RESULTS
    _ensure_axon_hooks()
    from concourse import bass_utils

    x = np.asarray(hidden_states, dtype=np.float32).reshape(T, H)
    gw = np.asarray(gate_weight, dtype=np.float32)
    ce = np.asarray(choosed_experts).astype(np.int64)
    w1 = np.asarray(w1, dtype=np.float32)
    w2 = np.asarray(w2, dtype=np.float32)

    # routing: stable sort of (token, k) pairs by expert
    flat = ce.reshape(-1)
    order = np.argsort(flat, kind="stable")
    counts = np.bincount(flat, minlength=E).astype(np.int64)
    starts = np.zeros(E + 1, dtype=np.int64)
    starts[1:] = np.cumsum(counts)

    C = max(512, int(-(-counts.max() // 64)) * 64)

    nc = _CACHE.get(C)
    if nc is None:
        nc = _build(C)
        _CACHE[C] = nc

    bf16 = ml_dtypes.bfloat16
    in_maps = []
    for e in range(E):
        p = order[starts[e]:starts[e + 1]]
        t_idx = p // K
        k_idx = p % K
        n_e = len(p)

        xT = np.zeros((H, C), dtype=bf16)
        xT[:, :n_e] = x[t_idx].T
        xT = np.ascontiguousarray(xT.reshape(8, P, C).transpose(1, 0, 2))

        w1_e = np.ascontiguousarray(
            w1[e].astype(bf16).reshape(8, P, F).transpose(1, 0, 2))
        w2_e = np.ascontiguousarray(
            w2[e].astype(bf16).reshape(32, P, H).transpose(1, 0, 2))

        coef = np.zeros((C,), dtype=np.float32)
        coef[:n_e] = gw[t_idx, k_idx]
        coef = np.ascontiguousarray(np.broadcast_to(coef[None, :], (P, C)))

        in_maps.append({"xT": xT, "w1": w1_e, "w2": w2_e, "coef": coef})

    res = bass_utils.run_bass_kernel_spmd(nc, in_maps, list(range(NCORES)),
                                          trace=TRACE)
    LAST_RESULTS = res

    # combine: place each pair's result back, then sum the K contributions
    ys = []
    for e in range(E):
        y = res.results[e]["y"]  # [P, 8, C] f32
        yT = y.transpose(1, 0, 2).reshape(H, C)
        n_e = int(counts[e])
        ys.append(yT[:, :n_e].T)
    all_pairs = np.concatenate(ys, axis=0)  # [T*K, H] in expert order
    out_pairs = np.empty((T * K, H), dtype=np.float32)
    out_pairs[order] = all_pairs
    return out_pairs.reshape(T, K, H).sum(axis=1)
